# revision 6
# baseline (speedup 1.0000x reference)
"""Two-layer GCN (GCNConv x2 + ReLU) on 8 Trainium2 NeuronCores — v2.

Strategy (src-sharded + ReduceScatter, both layers):
  - Nodes sharded 8 ways; core c owns shard c's rows and all edges whose
    SRC lies in shard c. Weights replicated.
  - norm factorization: norm(s,d) = dinv[s]*dinv[d]. Tables hold
    dinv[src]-prescaled features; dinv[dst] applied after the reduce.
    One-hots are pure 0/1.
  - L1: T1 = (dinv*x)_c @ W1 (sharded matmul). Per-edge rows gathered from
    the local T1 (int16 idx, 6272 rows), scattered into PSUM accumulators
    per global dst block via 0/1 one-hot matmuls, evacuated to a partial
    table [50176, 128] bf16, then ReduceScatter(add) -> agg1 shard.
  - post-RS: h-path per block via PSUM: I@agg + I@T1(self-loop) +
    (1/dinv)ox(b1) rank-1; T2 = dinv^2 * relu(.) (one DVE op), kept in
    SBUF + written to DRAM for L2 gathers.
  - L2: same edge schedule over T2, partials -> ReduceScatter -> agg2;
    out = (dinv*(agg2 + T2self)) @ W2 + b2, stored feature-major fp32.
  - RS is sliced [9, 4] groups so the first slice's collective overlaps
    the second slice's aggregation.
  - Host packs nodes into dst blocks bimodally (per-core edge count <=256
    or <=384) so the shared per-block chunk schedule wastes few slots.
"""
import sys
sys.path.insert(0, '/opt/trn_rl_repo')
import numpy as np
import concourse.bass as bass
import concourse.bacc as bacc
import concourse.mybir as mybir
import bass_rust
from concourse.tile import TileContext
from concourse.tile_rust import add_dep_helper
from concourse.bass_utils import run_bass_kernel_spmd

dt = mybir.dt

NCORES = 8
NSH = 6250          # nodes per shard
TS = 49             # dst blocks per shard
NSHP = TS * 128     # padded shard rows (6272)
SGB = 4             # blocks per store group
NGRP = 13           # groups per shard: 12 full (4 blocks) + 1 leftover (1 block)
SLICE_GROUPS = (9, 4)   # RS slicing in groups per shard
WINDOW = 8          # gather chunks per SWDGE window (device ring caps at 1024 descs)
SCRATCH = 16384     # dynamic dma scratch (device honors only the default)
TAB_DT = dt.bfloat16


def _np_dt(d):
    return mybir.dt.np(d)


def _groups_of_shard():
    """[(g, blocks, nrows_per_p)] — 12 groups of 4 blocks + 1 of 1 block."""
    out = []
    for g in range(12):
        out.append((g, list(range(4 * g, 4 * g + 4)), 4))
    out.append((12, [48], 1))
    return out


def _block_rowmap(l):
    """shard-local node index -> permuted table/partial row (vectorized)."""
    l = np.asarray(l)
    b = l // 128
    p = l % 128
    g = b // SGB
    s = b % SGB
    full = g * (SGB * 128) + p * SGB + s
    left = 12 * (SGB * 128) + p
    return np.where(b < 48, full, left)


# ---------------------------------------------------------------------------
def hoist_excess_waits(nc, max_attached=1):
    n_new = 0
    for f in nc.m.functions:
        for bb in f.blocks:
            insts = bb.instructions
            i = 0
            while i < len(insts):
                inst = insts[i]
                si = inst.sync_info
                if si is not None and inst.engine is not None:
                    waits = list(si.on_wait)
                    imm = [w for w in waits if w.wait_reg is None]
                    other = [w for w in waits if w.wait_reg is not None]
                    budget = max_attached - len(other)
                    if len(imm) > budget:
                        if budget > 0:
                            extra, keep = imm[:-budget], imm[-budget:]
                        else:
                            extra, keep = imm, []
                        for w in extra:
                            ev = mybir.InstEventSemaphore(
                                name=f"I-hoistw{n_new}", ins=[], outs=[])
                            ev.engine = inst.engine
                            h = bass_rust.SemaphoreHandle(name=w.ant_name, num=w.id)
                            bass_rust.wait_op(ev, h, w.wait_value, "sem-ge", True)
                            insts.insert(i, ev)
                            i += 1
                            n_new += 1
                        si.on_wait = other + keep
                i += 1
    return n_new


# ---------------------------------------------------------------------------
def _pack_blocks(edge_dst_core_counts, rng):
    """Assign shard-local nodes to 49 blocks, bimodal capacity.

    edge_dst_core_counts: [nsh_nodes, 8] in-degree of each shard node per
    src core. Returns perm: array of node positions -> block order, i.e.,
    an array `order` of length 6250 where order[i] = shard-local node id
    placed at padded slot i (blocks of 128, last block padded 22 wide).
    Strategy: sort by total indegree desc, snake-fill, then repair pass to
    push per-(core,block) sums under caps (256 for A blocks, 384 for B).
    """
    nsh = edge_dst_core_counts.shape[0]
    tot = edge_dst_core_counts.sum(1)
    # how many B (3-chunk) blocks: need sum caps to cover per-core totals
    per_core = edge_dst_core_counts.sum(0)          # [8]
    need = int(np.ceil((per_core.max() - 249 * TS) / 128.0))
    nB = int(np.clip(need + 2, 2, 20))
    caps = np.full(TS, 254)
    caps[:nB] = 382                                  # small safety margin
    order = np.argsort(-tot, kind='stable')
    # snake round-robin into blocks weighted by capacity
    slots = [[] for _ in range(TS)]
    sums = np.zeros((TS, NCORES), np.int64)
    sizes = np.full(TS, 128)
    sizes[TS - 1] = nsh - 128 * (TS - 1)
    cnt = np.zeros(TS, np.int64)
    # greedy: place each node in the block with most relative headroom
    headroom = caps.astype(np.float64).copy()
    for n in order:
        v = edge_dst_core_counts[n]
        # candidate blocks with space
        space = cnt < sizes
        util = (sums + v).max(1) / caps
        util[~space] = np.inf
        b = int(np.argmin(util))
        slots[b].append(n)
        sums[b] += v
        cnt[b] += 1
    out = np.concatenate([np.array(s, dtype=np.int64) for s in slots])
    m_b = np.maximum(np.ceil(sums.max(1) / 128.0).astype(np.int64), 1)
    return out, sums, m_b


def _prepare(x, edge_index, ncores):
    N, D = x.shape
    src0 = edge_index[0].astype(np.int64)
    dst0 = edge_index[1].astype(np.int64)
    # degree including the self-loops the reference adds
    deg = (np.bincount(np.concatenate([dst0, np.arange(N)]), minlength=N)
           .astype(np.float64))
    dinv = (1.0 / np.sqrt(np.maximum(deg, 1.0))).astype(np.float32)

    src_c = src0 // NSH

    # ---- per-shard dst block packing (bimodal) ----
    rng = np.random.default_rng(0)
    node_slot = np.zeros(N, np.int64)        # node -> padded slot in shard
    m_blocks = np.zeros((ncores, TS), np.int64)
    for d in range(ncores):
        n0, n1 = d * NSH, min(N, (d + 1) * NSH)
        nodes = np.arange(n0, n1)
        # per src-core indegree of each node in this shard
        mask = (dst0 >= n0) & (dst0 < n1)
        loc = dst0[mask] - n0
        sc = src_c[mask]
        cnts = np.zeros((n1 - n0, ncores), np.int64)
        np.add.at(cnts, (loc, sc), 1)
        order, sums, m_b = _pack_blocks(cnts, rng)
        # order[i] = shard-local node at padded position... place block by
        # block: block b gets order slice; last block is short (122 nodes)
        pos = 0
        for b in range(TS):
            size = 128 if b < TS - 1 else (n1 - n0) - 128 * (TS - 1)
            blk_nodes = order[pos:pos + size]
            node_slot[n0 + blk_nodes] = b * 128 + np.arange(size)
            pos += size
        m_blocks[d] = m_b

    # node -> global padded row / block / partition
    ncore_of = np.arange(N) // NSH
    loc_slot = node_slot                      # 0..6271 within shard
    blk_of = loc_slot // 128                  # 0..48
    p_of = loc_slot % 128
    table_row = _block_rowmap(loc_slot)       # permuted row within shard

    # ---- per-core edge schedules ----
    # global block order: slice-major, then (dstcore, block)
    s0 = SLICE_GROUPS[0]
    blk_slice = np.where(blk_of < 4 * s0, 0, 1)

    def block_order_key(d, b):
        sl = 0 if b < 4 * s0 else 1
        return (sl, d, b)

    all_blocks = sorted(
        ((d, b) for d in range(ncores) for b in range(TS)),
        key=lambda db: block_order_key(*db))
    # chunk layout shared by all cores
    chunk_of_block = {}
    chunks = []                                # (dstcore, block)
    for (d, b) in all_blocks:
        m = int(m_blocks[d, b])
        chunk_of_block[(d, b)] = (len(chunks), m)
        chunks += [(d, b)] * m
    NCH = len(chunks)
    slice_of_chunk = [0 if b < 4 * s0 else 1 for (d, b) in chunks]
    # windows must not span slices; taper the last chunks of each slice so
    # the compute drain behind the final gathers is short (RS fires sooner)
    slice_end = {}
    for k, sl in enumerate(slice_of_chunk):
        slice_end[sl] = k + 1
    windows = []                               # (start, count)
    i = 0
    while i < NCH:
        sl = slice_of_chunk[i]
        lim = WINDOW
        if slice_end[sl] - i <= 48:
            lim = 4
        j = i
        while j < NCH and j - i < lim and slice_of_chunk[j] == sl:
            j += 1
        windows.append((i, j - i))
        i = j

    # per-core edge data
    idx_np = np.zeros((ncores, 128, NCH * 8), np.int16)
    dstl_np = np.full((ncores, 128, NCH), 999.0, np.float32)
    for c in range(ncores):
        em = src_c == c
        es, ed = src0[em], dst0[em]
        flat_src = np.zeros(NCH * 128, np.int64)
        flat_dst = np.full(NCH * 128, 999.0, np.float32)
        # place edges of (d, b) into its chunk range
        key = ncore_of[ed] * TS + blk_of[ed]
        order = np.argsort(key, kind='stable')
        es, ed = es[order], ed[order]
        key = key[order]
        # boundaries per (d,b)
        uniq, starts = np.unique(key, return_index=True)
        starts = list(starts) + [len(es)]
        for ui, k in enumerate(uniq):
            d, b = divmod(int(k), TS)
            o, m = chunk_of_block[(d, b)]
            seg = slice(starts[ui], starts[ui + 1])
            n = starts[ui + 1] - starts[ui]
            assert n <= m * 128, (c, d, b, n, m)
            flat_src[o * 128:o * 128 + n] = table_row[es[seg]]
            flat_dst[o * 128:o * 128 + n] = p_of[ed[seg]]
        i16 = flat_src.astype(np.int16).reshape(-1, 16).T
        idx_np[c] = np.tile(i16, (8, 1))
        dstl_np[c] = flat_dst.reshape(NCH, 128).T

    # xT per core: [D, NSHP] bf16, columns BLOCK-major (b*128+p), dinv-scaled
    xT_np = np.zeros((ncores, D, NSHP), np.float32)
    for c in range(ncores):
        n0, n1 = c * NSH, min(N, (c + 1) * NSH)
        cols = loc_slot[n0:n1]
        xT_np[c][:, cols] = (x[n0:n1] * dinv[n0:n1, None]).T

    # per-core columns: dinv^2, dinv, 1/dinv at [128, TS] block layout
    dinv2c = np.zeros((ncores, 128, TS), np.float32)
    dinvc = np.zeros((ncores, 128, TS), np.float32)
    invdr = np.ones((ncores, 1, NSHP), np.float32)
    for c in range(ncores):
        n0, n1 = c * NSH, min(N, (c + 1) * NSH)
        b, p = blk_of[n0:n1], p_of[n0:n1]
        dv = dinv[n0:n1]
        dinv2c[c][p, b] = dv * dv
        dinvc[c][p, b] = dv
        # invd row indexed by BLOCK-major slot (b*128+p), used as rank-1 lhsT
        invdr[c][0, b * 128 + p] = 1.0 / dv

    iota = np.tile(np.arange(128, dtype=np.float32)[None, :], (128, 1)).copy()
    ident = np.eye(128, dtype=np.float32)

    # slice row extents (per shard, permuted rows are group-major so slices
    # are contiguous)
    srows = [SLICE_GROUPS[0] * 512, (12 - SLICE_GROUPS[0]) * 512 + 128]

    return dict(N=N, D=D, NCH=NCH, chunks=chunks, windows=windows,
                chunk_of_block=chunk_of_block, m_blocks=m_blocks,
                slice_of_chunk=slice_of_chunk, srows=srows,
                idx_np=idx_np, dstl_np=dstl_np, xT_np=xT_np,
                dinv2c=dinv2c, dinvc=dinvc, invdr=invdr,
                iota=iota, ident=ident, node_slot=node_slot,
                table_row=table_row)


# ---------------------------------------------------------------------------
def _build(cfg, F1, F2):
    D = cfg['D']
    NCH = cfg['NCH']
    chunks = cfg['chunks']
    windows = cfg['windows']
    srows = cfg['srows']
    KD = D // 128
    s0 = SLICE_GROUPS[0]

    nc = bacc.Bacc(None, target_bir_lowering=False,
                   dynamic_dma_scratch_size=SCRATCH)
    xT_d = nc.declare_dram_parameter("xT", [D, NSHP], TAB_DT, isOutput=False)
    W1_d = nc.declare_dram_parameter("W1", [D, F1], TAB_DT, isOutput=False)
    b1_d = nc.declare_dram_parameter("b1r", [1, F1], TAB_DT, isOutput=False)
    W2_d = nc.declare_dram_parameter("W2", [F1, F2], TAB_DT, isOutput=False)
    b2_d = nc.declare_dram_parameter("b2", [F2, 1], dt.float32, isOutput=False)
    iota_d = nc.declare_dram_parameter("iota", [128, 128], TAB_DT, isOutput=False)
    ident_d = nc.declare_dram_parameter("ident", [128, 128], TAB_DT, isOutput=False)
    invd_d = nc.declare_dram_parameter("invd", [1, NSHP], TAB_DT, isOutput=False)
    dinv2_d = nc.declare_dram_parameter("dinv2c", [128, TS], dt.float32, isOutput=False)
    dinvc_d = nc.declare_dram_parameter("dinvc", [128, TS], dt.float32, isOutput=False)
    idx_d = nc.declare_dram_parameter("idx", [128, NCH * 8], dt.int16, isOutput=False)
    dstl_d = nc.declare_dram_parameter("dstl", [128, NCH], dt.float32, isOutput=False)
    out_d = nc.declare_dram_parameter("outT", [F2, NSHP], dt.float32, isOutput=True)

    T1_d = nc.dram_tensor("T1", [NSHP, F1], TAB_DT)
    T2_d = nc.dram_tensor("T2", [NSHP, F1], TAB_DT)
    PART_DT = [TAB_DT, TAB_DT]
    p_d = [[nc.dram_tensor(f"p{ly}s{k}", [NCORES * srows[k], F1], PART_DT[ly])
            for k in range(2)] for ly in range(2)]
    agg_d = [[nc.dram_tensor(f"agg{ly}s{k}", [srows[k], F1], PART_DT[ly])
              for k in range(2)] for ly in range(2)]

    groups = _groups_of_shard()

    with TileContext(nc) as tc:
        with (
            tc.tile_pool(name="const", bufs=1) as cp,
            tc.tile_pool(name="tab", bufs=1) as tabp,
            tc.tile_pool(name="xw", bufs=5) as xp,
            tc.tile_pool(name="gat", bufs=13) as gp,
            tc.tile_pool(name="oh", bufs=20) as ohp,
            tc.tile_pool(name="evac", bufs=8) as evp,
            tc.tile_pool(name="post", bufs=4) as pp,
        ):
            # ---- constants ----
            iota_t = cp.tile([128, 128], TAB_DT, tag="iota")
            nc.sync.dma_start(iota_t[:], iota_d[:])
            ident_t = cp.tile([128, 128], TAB_DT, tag="ident")
            nc.sync.dma_start(ident_t[:], ident_d[:])
            b1r_t = cp.tile([1, F1], TAB_DT, tag="b1r")
            nc.sync.dma_start(b1r_t[:], b1_d[:])
            b2_t = cp.tile([F2, 1], dt.float32, tag="b2")
            nc.sync.dma_start(b2_t[:], b2_d[:])
            W1_t = cp.tile([128, KD, F1], TAB_DT, tag="W1")
            nc.sync.dma_start(W1_t[:], W1_d[:].rearrange("(k p) f -> p k f", p=128))
            W2_t = cp.tile([F1, F2], TAB_DT, tag="W2")
            nc.sync.dma_start(W2_t[:], W2_d[:])
            invd_t = cp.tile([1, NSHP], TAB_DT, tag="invd")
            nc.sync.dma_start(invd_t[:], invd_d[:])
            dinv2_t = cp.tile([128, TS], dt.float32, tag="dinv2")
            nc.sync.dma_start(dinv2_t[:], dinv2_d[:])
            dinvc_t = cp.tile([128, TS], dt.float32, tag="dinvc")
            nc.sync.dma_start(dinvc_t[:], dinvc_d[:])
            # node-block-major resident tables
            T1_s = tabp.tile([128, TS, F1], TAB_DT, tag="T1s")
            T2_s = tabp.tile([128, TS, F1], TAB_DT, tag="T2s")

            # ---- phase A: T1 = xT' @ W1 (sharded) ----
            t1_writes = []
            with tc.tile_pool(name="xwps", bufs=3, space="PSUM") as xpp:
                for (g, blocks, nrp) in groups:
                    r0 = g * 512
                    ncols = len(blocks) * 128
                    xt = xp.tile([128, KD, 512], TAB_DT, tag="xt")
                    nc.sync.dma_start(
                        xt[:, :, 0:ncols],
                        xT_d[:, :].rearrange("(k p) n -> p k n", p=128)
                        [:, :, r0:r0 + ncols])
                    ps = xpp.tile([128, 512], dt.float32, tag="xwps")
                    for si, b in enumerate(blocks):
                        for k in range(KD):
                            nc.tensor.matmul(
                                ps[:, si * 128:(si + 1) * 128],
                                xt[:, k, si * 128:(si + 1) * 128],
                                W1_t[:, k, :],
                                start=(k == 0), stop=(k == KD - 1))
                    nc.scalar.activation(
                        T1_s[:, blocks[0]:blocks[0] + len(blocks), :]
                        .rearrange("p s f -> p (s f)"),
                        ps[:, 0:ncols],
                        mybir.ActivationFunctionType.Copy)
                    w = nc.sync.dma_start(
                        T1_d[r0:r0 + len(blocks) * 128, :]
                        .rearrange("(p s) f -> p s f", s=nrp),
                        T1_s[:, blocks[0]:blocks[0] + len(blocks), :])
                    t1_writes.append(w)

            # metadata loads after the xw stream so T1 lands sooner
            idx_t = cp.tile([128, NCH * 8], dt.int16, tag="idx")
            nc.sync.dma_start(idx_t[:], idx_d[:])
            dstl_t = cp.tile([128, NCH], dt.float32, tag="dstl")
            nc.sync.dma_start(dstl_t[:], dstl_d[:])

            # ---- shared aggregation pass ----
            def agg_pass(tab_d, tab_writes, part, lyr, dve_after=None,
                         pe_after=None):
                rs_insts = [None, None]
                last_ops = {'pe': None, 'dve': None, 'act': None}
                block_done = {}
                p_writes = [[], []]
                # per-chunk tile slot bookkeeping
                win_of_chunk = {}
                for wi, (o, m) in enumerate(windows):
                    for k in range(o, o + m):
                        win_of_chunk[k] = (wi, k - o)
                with tc.tile_pool(name=f"aggps{lyr}", bufs=8, space="PSUM") as app:
                    gts = {}
                    accs = {}
                    evac_sel = 0
                    ch = 0
                    for wi, (o, m) in enumerate(windows):
                        gt = gp.tile([128, WINDOW, F1], TAB_DT, tag="gat")
                        gts[wi] = gt
                        gi = nc.gpsimd.dma_gather(
                            gt[:, 0:m, :], tab_d[:],
                            idx_t[:, o * 8:(o + m) * 8],
                            num_idxs=m * 128, num_idxs_reg=m * 128,
                            elem_size=F1)
                        for dep in tab_writes:
                            add_dep_helper(gi.ins, dep.ins, reason="table dep")
                        for k in range(o, o + m):
                            d, b = chunks[k]
                            grp = min(b // SGB, 12)
                            key = (d, grp)
                            if key not in accs:
                                accs[key] = app.tile([128, 512], dt.float32,
                                                     name=f"acc{lyr}_{d}_{grp}",
                                                     tag="acc")
                            acc = accs[key]
                            si = b - grp * SGB
                            oh = ohp.tile([128, 128], TAB_DT, tag="oh")
                            ohi = nc.vector.tensor_scalar(
                                oh[:], iota_t[:], dstl_t[:, k:k + 1], None,
                                mybir.AluOpType.is_equal)
                            if dve_after is not None:
                                add_dep_helper(ohi.ins, dve_after.ins,
                                               reason="order after prev phase")
                                dve_after = None
                            last_ops['dve'] = ohi
                            co, cm = cfg['chunk_of_block'][(d, b)]
                            if pe_after is not None:
                                pass
                            last_ops['pe'] = nc.tensor.matmul(
                                acc[:, si * 128:(si + 1) * 128],
                                oh[:], gt[:, k - o, :],
                                start=(k == co), stop=(k == co + cm - 1))
                            if k == co + cm - 1:
                                block_done.setdefault(key, []).append(b)
                                grp_blocks = groups[grp][1]
                                if len(block_done[key]) == len(grp_blocks):
                                    nrp = groups[grp][2]
                                    ncols = len(grp_blocks) * 128
                                    st = evp.tile([128, 512], PART_DT[lyr],
                                                  tag=f"pstage{lyr}")
                                    last_ops['act'] = nc.scalar.activation(
                                        st[:, 0:ncols], acc[:, 0:ncols],
                                        mybir.ActivationFunctionType.Copy)
                                    sl = 0 if grp < s0 else 1
                                    r0 = (grp * 512 if sl == 0
                                          else grp * 512 - srows[0])
                                    pw = nc.sync.dma_start(
                                        part[sl][d * srows[sl] + r0:
                                                 d * srows[sl] + r0 + ncols, :]
                                        .rearrange("(p s) f -> p s f", s=nrp),
                                        st[:, 0:ncols]
                                        .rearrange("p (s f) -> p s f", s=nrp))
                                    p_writes[sl].append(pw)
                        # slice end -> fire RS
                        nxt = windows[wi + 1] if wi + 1 < len(windows) else None
                        cur_sl = cfg['slice_of_chunk'][o]
                        if nxt is None or cfg['slice_of_chunk'][nxt[0]] != cur_sl:
                            cc = nc.gpsimd.collective_compute(
                                "ReduceScatter", mybir.AluOpType.add,
                                replica_groups=[list(range(NCORES))],
                                ins=[part[cur_sl][:]],
                                outs=[agg_d[lyr][cur_sl][:]])
                            for w in p_writes[cur_sl]:
                                add_dep_helper(cc.ins, w.ins,
                                               reason="rs reads partials")
                            rs_insts[cur_sl] = cc
                return rs_insts, last_ops

            rs1, l1_last = agg_pass(T1_d, t1_writes, p_d[0], 0)

            # ---- phase C: agg1 -> T2 ----
            # per-slice block-major loads of the RS output: full 512-row
            # groups land via (g p s) -> p (g s); the leftover 128-row group
            # is a separate 1-block dma.
            def load_agg_slices(agg, rs, tag):
                tiles = {}
                adt = agg[0].dtype if hasattr(agg[0], 'dtype') else TAB_DT
                for sl in range(2):
                    gfull = SLICE_GROUPS[sl] - (1 if sl == 1 else 0)
                    raw = tabp.tile([128, gfull, SGB, F1], adt,
                                    tag=f"{tag}r{sl}")
                    ld = nc.scalar.dma_start(
                        raw[:],
                        agg[sl][0:gfull * 512, :]
                        .rearrange("(g p s) f -> p g s f", p=128, s=SGB))
                    add_dep_helper(ld.ins, rs[sl].ins, reason="load after rs")
                    if adt != TAB_DT:
                        t = tabp.tile([128, gfull, SGB, F1], TAB_DT,
                                      tag=f"{tag}{sl}")
                        nc.scalar.activation(
                            t[:].rearrange("p g s f -> p (g s f)"),
                            raw[:].rearrange("p g s f -> p (g s f)"),
                            mybir.ActivationFunctionType.Copy)
                    else:
                        t = raw
                    tl = None
                    if sl == 1:
                        rawl = tabp.tile([128, 1, F1], adt, tag=f"{tag}rL")
                        ld2 = nc.scalar.dma_start(
                            rawl[:],
                            agg[sl][gfull * 512:gfull * 512 + 128, :]
                            .rearrange("(p s) f -> p s f", s=1))
                        add_dep_helper(ld2.ins, rs[sl].ins,
                                       reason="load after rs")
                        if adt != TAB_DT:
                            tl = tabp.tile([128, 1, F1], TAB_DT,
                                           tag=f"{tag}L")
                            nc.scalar.activation(
                                tl[:].rearrange("p s f -> p (s f)"),
                                rawl[:].rearrange("p s f -> p (s f)"),
                                mybir.ActivationFunctionType.Copy)
                        else:
                            tl = rawl
                    tiles[sl] = (t, tl)

                def blk_ap(b):
                    sl = 0 if b < s0 * SGB else 1
                    off = b - (0 if sl == 0 else s0 * SGB)
                    t, tl = tiles[sl]
                    if sl == 1 and off == (SLICE_GROUPS[1] - 1) * SGB:
                        return tl[:, 0, :]
                    return t[:, off // SGB, off % SGB, :]
                return blk_ap

            t2_writes = []
            with tc.tile_pool(name="cps", bufs=4, space="PSUM") as cpp:
                blk1 = load_agg_slices(agg_d[0], rs1, "agg1s")
                for (g, blocks, nrp) in groups:
                    ncols = len(blocks) * 128
                    for si, b in enumerate(blocks):
                        ps = cpp.tile([128, F1], dt.float32, tag="cps")
                        q = ps[:]
                        m1 = nc.tensor.matmul(q, ident_t[:], blk1(b),
                                              start=True, stop=False)
                        add_dep_helper(m1.ins, l1_last['pe'].ins,
                                       reason="order after L1 PE tail")
                        nc.tensor.matmul(q, ident_t[:], T1_s[:, b, :],
                                         start=False, stop=False)
                        nc.tensor.matmul(q,
                                         invd_t[:, b * 128:(b + 1) * 128],
                                         b1r_t[:], start=False, stop=True)
                        sc = nc.scalar.activation(
                            T2_s[:, b, :], q,
                            mybir.ActivationFunctionType.Relu,
                            scale=dinv2_t[:, b:b + 1])
                        add_dep_helper(sc.ins, l1_last['act'].ins,
                                       reason="order after L1 ACT tail")
                        c_last_dve = sc
                    w = nc.scalar.dma_start(
                        T2_d[g * 512:g * 512 + ncols, :]
                        .rearrange("(p s) f -> p s f", s=nrp),
                        T2_s[:, blocks[0]:blocks[0] + len(blocks), :])
                    t2_writes.append(w)

            rs2, l2_last = agg_pass(T2_d, t2_writes, p_d[1], 1,
                                    dve_after=c_last_dve)

            # ---- phase E: out = (dinv*(agg2 + T2self)) @ W2 + b2 ----
            with (
                tc.tile_pool(name="eps", bufs=2, space="PSUM") as epp,
                tc.tile_pool(name="ops", bufs=4, space="PSUM") as opp,
            ):
                blk2 = load_agg_slices(agg_d[1], rs2, "agg2s")
                for (g, blocks, nrp) in groups:
                    ncols = len(blocks) * 128
                    ost = evp.tile([F2, SGB, 128], dt.float32, tag="ostage")
                    for si, b in enumerate(blocks):
                        ps = epp.tile([128, F1], dt.float32, tag="eps")
                        q = ps[:]
                        m1 = nc.tensor.matmul(q, ident_t[:], blk2(b),
                                              start=True, stop=False)
                        add_dep_helper(m1.ins, l2_last['pe'].ins,
                                       reason="order after L2 PE tail")
                        nc.tensor.matmul(q, ident_t[:], T2_s[:, b, :],
                                         start=False, stop=True)
                        z = pp.tile([128, F1], TAB_DT, tag="z")
                        zo = nc.scalar.activation(
                            z[:], q, mybir.ActivationFunctionType.Identity,
                            scale=dinvc_t[:, b:b + 1])
                        add_dep_helper(zo.ins, l2_last['act'].ins,
                                       reason="order after L2 ACT tail")
                        pt = epp.tile([128, 128], TAB_DT, tag="psT")
                        nc.tensor.transpose(pt[:], z[:], ident_t[:])
                        zT = pp.tile([F1, 128], TAB_DT, tag="zT")
                        nc.vector.tensor_copy(zT[:], pt[:])
                        po = opp.tile([F2, 128], dt.float32, tag="ops")
                        nc.tensor.matmul(po[:], W2_t[:], zT[:],
                                         start=True, stop=True)
                        nc.scalar.activation(
                            ost[:, si, :], po[:],
                            mybir.ActivationFunctionType.Identity,
                            bias=b2_t[:, 0:1], scale=1.0)
                    nc.scalar.dma_start(
                        out_d[:, g * 512:g * 512 + ncols]
                        .rearrange("f (s n) -> f s n", s=nrp),
                        ost[:, 0:len(blocks), :])

    if not nc.is_finalized():
        nc.finalize()
    hoist_excess_waits(nc)
    return nc


# ---------------------------------------------------------------------------
cfg = None  # set by _kernel_impl for _build's closure use


def _kernel_impl(x, edge_index, W1, b1, W2, b2, ncores=NCORES):
    global cfg
    x = np.asarray(x, dtype=np.float32)
    edge_index = np.asarray(edge_index)
    W1 = np.asarray(W1, dtype=np.float32)
    b1 = np.asarray(b1, dtype=np.float32)
    W2 = np.asarray(W2, dtype=np.float32)
    b2 = np.asarray(b2, dtype=np.float32)
    N, D = x.shape
    F1 = W1.shape[1]
    F2 = W2.shape[1]

    cfg = _prepare(x, edge_index, ncores)
    nc = _build(cfg, F1, F2)

    tabnp = _np_dt(TAB_DT)
    in_maps = []
    for c in range(ncores):
        in_maps.append({
            "xT": cfg['xT_np'][c].astype(tabnp),
            "W1": W1.astype(tabnp),
            "b1r": b1.reshape(1, F1).astype(tabnp),
            "W2": W2.astype(tabnp),
            "b2": b2.reshape(F2, 1).astype(np.float32),
            "iota": cfg['iota'].astype(tabnp),
            "ident": cfg['ident'].astype(tabnp),
            "invd": cfg['invdr'][c].astype(tabnp),
            "dinv2c": cfg['dinv2c'][c],
            "dinvc": cfg['dinvc'][c],
            "idx": cfg['idx_np'][c],
            "dstl": cfg['dstl_np'][c],
        })
    res = run_bass_kernel_spmd(nc, in_maps, list(range(ncores)))

    out = np.empty((N, F2), np.float32)
    for c in range(ncores):
        oT = res.results[c]["outT"]          # [F2, NSHP], block-major cols
        n0, n1 = c * NSH, min(N, (c + 1) * NSH)
        cols = cfg['node_slot'][n0:n1]
        out[n0:n1] = oT[:, cols].T
    return out, res, nc, cfg


def kernel(x, edge_index, W1, b1, W2, b2):
    out, _, _, _ = _kernel_impl(x, edge_index, W1, b1, W2, b2)
    return out



# revision 10
# speedup vs baseline: 1.0390x; 1.0390x over previous
"""Two-layer GCN (GCNConv x2 + ReLU) on 8 Trainium2 NeuronCores — v2.

Strategy (src-sharded + ReduceScatter, both layers):
  - Nodes sharded 8 ways; core c owns shard c's rows and all edges whose
    SRC lies in shard c. Weights replicated.
  - norm factorization: norm(s,d) = dinv[s]*dinv[d]. Tables hold
    dinv[src]-prescaled features; dinv[dst] applied after the reduce.
    One-hots are pure 0/1.
  - L1: T1 = (dinv*x)_c @ W1 (sharded matmul). Per-edge rows gathered from
    the local T1 (int16 idx, 6272 rows), scattered into PSUM accumulators
    per global dst block via 0/1 one-hot matmuls, evacuated to a partial
    table [50176, 128] bf16, then ReduceScatter(add) -> agg1 shard.
  - post-RS: h-path per block via PSUM: I@agg + I@T1(self-loop) +
    (1/dinv)ox(b1) rank-1; T2 = dinv^2 * relu(.) (one DVE op), kept in
    SBUF + written to DRAM for L2 gathers.
  - L2: same edge schedule over T2, partials -> ReduceScatter -> agg2;
    out = (dinv*(agg2 + T2self)) @ W2 + b2, stored feature-major fp32.
  - RS is sliced [9, 4] groups so the first slice's collective overlaps
    the second slice's aggregation.
  - Host packs nodes into dst blocks bimodally (per-core edge count <=256
    or <=384) so the shared per-block chunk schedule wastes few slots.
"""
import sys
sys.path.insert(0, '/opt/trn_rl_repo')
import numpy as np
import concourse.bass as bass
import concourse.bacc as bacc
import concourse.mybir as mybir
import bass_rust
from concourse.tile import TileContext
from concourse.tile_rust import add_dep_helper
from concourse.bass_utils import run_bass_kernel_spmd

dt = mybir.dt

NCORES = 8
NSH = 6250          # nodes per shard
TS = 49             # dst blocks per shard
NSHP = TS * 128     # padded shard rows (6272)
SGB = 4             # blocks per store group
NGRP = 13           # groups per shard: 12 full (4 blocks) + 1 leftover (1 block)
SLICE_GROUPS = (9, 4)   # RS slicing in groups per shard
WINDOW = 8          # gather chunks per SWDGE window (device ring caps at 1024 descs)
SCRATCH = 16384     # dynamic dma scratch (device honors only the default)
TAB_DT = dt.bfloat16


def _np_dt(d):
    return mybir.dt.np(d)


def _groups_of_shard():
    """[(g, blocks, nrows_per_p)] — 12 groups of 4 blocks + 1 of 1 block."""
    out = []
    for g in range(12):
        out.append((g, list(range(4 * g, 4 * g + 4)), 4))
    out.append((12, [48], 1))
    return out


def _block_rowmap(l):
    """shard-local node index -> permuted table/partial row (vectorized)."""
    l = np.asarray(l)
    b = l // 128
    p = l % 128
    g = b // SGB
    s = b % SGB
    full = g * (SGB * 128) + p * SGB + s
    left = 12 * (SGB * 128) + p
    return np.where(b < 48, full, left)


# ---------------------------------------------------------------------------
def hoist_excess_waits(nc, max_attached=1):
    n_new = 0
    for f in nc.m.functions:
        for bb in f.blocks:
            insts = bb.instructions
            i = 0
            while i < len(insts):
                inst = insts[i]
                si = inst.sync_info
                if si is not None and inst.engine is not None:
                    waits = list(si.on_wait)
                    imm = [w for w in waits if w.wait_reg is None]
                    other = [w for w in waits if w.wait_reg is not None]
                    budget = max_attached - len(other)
                    if len(imm) > budget:
                        if budget > 0:
                            extra, keep = imm[:-budget], imm[-budget:]
                        else:
                            extra, keep = imm, []
                        for w in extra:
                            ev = mybir.InstEventSemaphore(
                                name=f"I-hoistw{n_new}", ins=[], outs=[])
                            ev.engine = inst.engine
                            h = bass_rust.SemaphoreHandle(name=w.ant_name, num=w.id)
                            bass_rust.wait_op(ev, h, w.wait_value, "sem-ge", True)
                            insts.insert(i, ev)
                            i += 1
                            n_new += 1
                        si.on_wait = other + keep
                i += 1
    return n_new


# ---------------------------------------------------------------------------
def _pack_blocks(edge_dst_core_counts, rng):
    """Assign shard-local nodes to 49 blocks, bimodal capacity.

    edge_dst_core_counts: [nsh_nodes, 8] in-degree of each shard node per
    src core. Returns perm: array of node positions -> block order, i.e.,
    an array `order` of length 6250 where order[i] = shard-local node id
    placed at padded slot i (blocks of 128, last block padded 22 wide).
    Strategy: sort by total indegree desc, snake-fill, then repair pass to
    push per-(core,block) sums under caps (256 for A blocks, 384 for B).
    """
    nsh = edge_dst_core_counts.shape[0]
    tot = edge_dst_core_counts.sum(1)
    # how many B (3-chunk) blocks: need sum caps to cover per-core totals
    per_core = edge_dst_core_counts.sum(0)          # [8]
    need = int(np.ceil((per_core.max() - 249 * TS) / 128.0))
    nB = int(np.clip(need + 2, 2, 20))
    caps = np.full(TS, 254)
    caps[:nB] = 382                                  # small safety margin
    order = np.argsort(-tot, kind='stable')
    # snake round-robin into blocks weighted by capacity
    slots = [[] for _ in range(TS)]
    sums = np.zeros((TS, NCORES), np.int64)
    sizes = np.full(TS, 128)
    sizes[TS - 1] = nsh - 128 * (TS - 1)
    cnt = np.zeros(TS, np.int64)
    # greedy: place each node in the block with most relative headroom
    headroom = caps.astype(np.float64).copy()
    for n in order:
        v = edge_dst_core_counts[n]
        # candidate blocks with space
        space = cnt < sizes
        util = (sums + v).max(1) / caps
        util[~space] = np.inf
        b = int(np.argmin(util))
        slots[b].append(n)
        sums[b] += v
        cnt[b] += 1
    out = np.concatenate([np.array(s, dtype=np.int64) for s in slots])
    m_b = np.maximum(np.ceil(sums.max(1) / 128.0).astype(np.int64), 1)
    return out, sums, m_b


def _prepare(x, edge_index, ncores):
    N, D = x.shape
    src0 = edge_index[0].astype(np.int64)
    dst0 = edge_index[1].astype(np.int64)
    # degree including the self-loops the reference adds
    deg = (np.bincount(np.concatenate([dst0, np.arange(N)]), minlength=N)
           .astype(np.float64))
    dinv = (1.0 / np.sqrt(np.maximum(deg, 1.0))).astype(np.float32)

    src_c = src0 // NSH

    # ---- per-shard dst block packing (bimodal) ----
    rng = np.random.default_rng(0)
    node_slot = np.zeros(N, np.int64)        # node -> padded slot in shard
    m_blocks = np.zeros((ncores, TS), np.int64)
    for d in range(ncores):
        n0, n1 = d * NSH, min(N, (d + 1) * NSH)
        nodes = np.arange(n0, n1)
        # per src-core indegree of each node in this shard
        mask = (dst0 >= n0) & (dst0 < n1)
        loc = dst0[mask] - n0
        sc = src_c[mask]
        cnts = np.zeros((n1 - n0, ncores), np.int64)
        np.add.at(cnts, (loc, sc), 1)
        order, sums, m_b = _pack_blocks(cnts, rng)
        # order[i] = shard-local node at padded position... place block by
        # block: block b gets order slice; last block is short (122 nodes)
        pos = 0
        for b in range(TS):
            size = 128 if b < TS - 1 else (n1 - n0) - 128 * (TS - 1)
            blk_nodes = order[pos:pos + size]
            node_slot[n0 + blk_nodes] = b * 128 + np.arange(size)
            pos += size
        m_blocks[d] = m_b

    # node -> global padded row / block / partition
    ncore_of = np.arange(N) // NSH
    loc_slot = node_slot                      # 0..6271 within shard
    blk_of = loc_slot // 128                  # 0..48
    p_of = loc_slot % 128
    table_row = _block_rowmap(loc_slot)       # permuted row within shard

    # ---- per-core edge schedules ----
    # global block order: slice-major, then (dstcore, block)
    s0 = SLICE_GROUPS[0]
    blk_slice = np.where(blk_of < 4 * s0, 0, 1)

    def block_order_key(d, b):
        sl = 0 if b < 4 * s0 else 1
        return (sl, d, b)

    all_blocks = sorted(
        ((d, b) for d in range(ncores) for b in range(TS)),
        key=lambda db: block_order_key(*db))
    # chunk layout shared by all cores
    chunk_of_block = {}
    chunks = []                                # (dstcore, block)
    for (d, b) in all_blocks:
        m = int(m_blocks[d, b])
        chunk_of_block[(d, b)] = (len(chunks), m)
        chunks += [(d, b)] * m
    NCH = len(chunks)
    slice_of_chunk = [0 if b < 4 * s0 else 1 for (d, b) in chunks]
    # windows must not span slices; taper the last chunks of each slice so
    # the compute drain behind the final gathers is short (RS fires sooner)
    slice_end = {}
    for k, sl in enumerate(slice_of_chunk):
        slice_end[sl] = k + 1
    windows = []                               # (start, count)
    i = 0
    while i < NCH:
        sl = slice_of_chunk[i]
        lim = WINDOW
        if slice_end[sl] - i <= 48:
            lim = 4
        j = i
        while j < NCH and j - i < lim and slice_of_chunk[j] == sl:
            j += 1
        windows.append((i, j - i))
        i = j

    # per-core edge data
    idx_np = np.zeros((ncores, 128, NCH * 8), np.int16)
    dstl_np = np.full((ncores, 128, NCH), 999.0, np.float32)
    for c in range(ncores):
        em = src_c == c
        es, ed = src0[em], dst0[em]
        flat_src = np.zeros(NCH * 128, np.int64)
        flat_dst = np.full(NCH * 128, 999.0, np.float32)
        # place edges of (d, b) into its chunk range
        key = ncore_of[ed] * TS + blk_of[ed]
        order = np.argsort(key, kind='stable')
        es, ed = es[order], ed[order]
        key = key[order]
        # boundaries per (d,b)
        uniq, starts = np.unique(key, return_index=True)
        starts = list(starts) + [len(es)]
        for ui, k in enumerate(uniq):
            d, b = divmod(int(k), TS)
            o, m = chunk_of_block[(d, b)]
            seg = slice(starts[ui], starts[ui + 1])
            n = starts[ui + 1] - starts[ui]
            assert n <= m * 128, (c, d, b, n, m)
            flat_src[o * 128:o * 128 + n] = table_row[es[seg]]
            flat_dst[o * 128:o * 128 + n] = p_of[ed[seg]]
        i16 = flat_src.astype(np.int16).reshape(-1, 16).T
        idx_np[c] = np.tile(i16, (8, 1))
        dstl_np[c] = flat_dst.reshape(NCH, 128).T

    # xT per core: [D, NSHP] bf16, columns BLOCK-major (b*128+p), dinv-scaled
    xT_np = np.zeros((ncores, D, NSHP), np.float32)
    for c in range(ncores):
        n0, n1 = c * NSH, min(N, (c + 1) * NSH)
        cols = loc_slot[n0:n1]
        xT_np[c][:, cols] = (x[n0:n1] * dinv[n0:n1, None]).T

    # per-core columns: dinv^2, dinv, 1/dinv at [128, TS] block layout
    dinv2c = np.zeros((ncores, 128, TS), np.float32)
    dinvc = np.zeros((ncores, 128, TS), np.float32)
    invdr = np.ones((ncores, 1, NSHP), np.float32)
    for c in range(ncores):
        n0, n1 = c * NSH, min(N, (c + 1) * NSH)
        b, p = blk_of[n0:n1], p_of[n0:n1]
        dv = dinv[n0:n1]
        dinv2c[c][p, b] = dv * dv
        dinvc[c][p, b] = dv
        # invd row indexed by BLOCK-major slot (b*128+p), used as rank-1 lhsT
        invdr[c][0, b * 128 + p] = 1.0 / dv

    iota = np.tile(np.arange(128, dtype=np.float32)[None, :], (128, 1)).copy()
    ident = np.eye(128, dtype=np.float32)

    # slice row extents (per shard, permuted rows are group-major so slices
    # are contiguous)
    srows = [SLICE_GROUPS[0] * 512, (12 - SLICE_GROUPS[0]) * 512 + 128]

    return dict(N=N, D=D, NCH=NCH, chunks=chunks, windows=windows,
                chunk_of_block=chunk_of_block, m_blocks=m_blocks,
                slice_of_chunk=slice_of_chunk, srows=srows,
                idx_np=idx_np, dstl_np=dstl_np, xT_np=xT_np,
                dinv2c=dinv2c, dinvc=dinvc, invdr=invdr,
                iota=iota, ident=ident, node_slot=node_slot,
                table_row=table_row)


# ---------------------------------------------------------------------------
def _build(cfg, F1, F2):
    D = cfg['D']
    NCH = cfg['NCH']
    chunks = cfg['chunks']
    windows = cfg['windows']
    srows = cfg['srows']
    KD = D // 128
    s0 = SLICE_GROUPS[0]

    nc = bacc.Bacc(None, target_bir_lowering=False,
                   dynamic_dma_scratch_size=SCRATCH)
    xT_d = nc.declare_dram_parameter("xT", [D, NSHP], TAB_DT, isOutput=False)
    W1_d = nc.declare_dram_parameter("W1", [D, F1], TAB_DT, isOutput=False)
    b1_d = nc.declare_dram_parameter("b1r", [1, F1], TAB_DT, isOutput=False)
    W2_d = nc.declare_dram_parameter("W2", [F1, F2], TAB_DT, isOutput=False)
    b2_d = nc.declare_dram_parameter("b2", [F2, 1], dt.float32, isOutput=False)
    iota_d = nc.declare_dram_parameter("iota", [128, 128], TAB_DT, isOutput=False)
    ident_d = nc.declare_dram_parameter("ident", [128, 128], TAB_DT, isOutput=False)
    invd_d = nc.declare_dram_parameter("invd", [1, NSHP], TAB_DT, isOutput=False)
    dinv2_d = nc.declare_dram_parameter("dinv2c", [128, TS], dt.float32, isOutput=False)
    dinvc_d = nc.declare_dram_parameter("dinvc", [128, TS], dt.float32, isOutput=False)
    idx_d = nc.declare_dram_parameter("idx", [128, NCH * 8], dt.int16, isOutput=False)
    dstl_d = nc.declare_dram_parameter("dstl", [128, NCH], dt.float32, isOutput=False)
    out_d = nc.declare_dram_parameter("outT", [F2, NSHP], dt.float32, isOutput=True)

    T1_d = nc.dram_tensor("T1", [NSHP, F1], TAB_DT)
    T2_d = nc.dram_tensor("T2", [NSHP, F1], TAB_DT)
    PART_DT = [TAB_DT, TAB_DT]
    p_d = [[nc.dram_tensor(f"p{ly}s{k}", [NCORES * srows[k], F1], PART_DT[ly])
            for k in range(2)] for ly in range(2)]
    agg_d = [[nc.dram_tensor(f"agg{ly}s{k}", [srows[k], F1], PART_DT[ly])
              for k in range(2)] for ly in range(2)]

    groups = _groups_of_shard()

    with TileContext(nc) as tc:
        with (
            tc.tile_pool(name="const", bufs=1) as cp,
            tc.tile_pool(name="tab", bufs=1) as tabp,
            tc.tile_pool(name="xw", bufs=5) as xp,
            tc.tile_pool(name="gat", bufs=13) as gp,
            tc.tile_pool(name="oh", bufs=20) as ohp,
            tc.tile_pool(name="evac", bufs=8) as evp,
            tc.tile_pool(name="post", bufs=4) as pp,
        ):
            # ---- constants ----
            iota_t = cp.tile([128, 128], TAB_DT, tag="iota")
            nc.sync.dma_start(iota_t[:], iota_d[:])
            ident_t = cp.tile([128, 128], TAB_DT, tag="ident")
            nc.sync.dma_start(ident_t[:], ident_d[:])
            b1r_t = cp.tile([1, F1], TAB_DT, tag="b1r")
            nc.sync.dma_start(b1r_t[:], b1_d[:])
            b2_t = cp.tile([F2, 1], dt.float32, tag="b2")
            nc.sync.dma_start(b2_t[:], b2_d[:])
            W1_t = cp.tile([128, KD, F1], TAB_DT, tag="W1")
            nc.sync.dma_start(W1_t[:], W1_d[:].rearrange("(k p) f -> p k f", p=128))
            W2_t = cp.tile([F1, F2], TAB_DT, tag="W2")
            nc.sync.dma_start(W2_t[:], W2_d[:])
            invd_t = cp.tile([1, NSHP], TAB_DT, tag="invd")
            nc.sync.dma_start(invd_t[:], invd_d[:])
            dinv2_t = cp.tile([128, TS], dt.float32, tag="dinv2")
            nc.sync.dma_start(dinv2_t[:], dinv2_d[:])
            dinvc_t = cp.tile([128, TS], dt.float32, tag="dinvc")
            nc.sync.dma_start(dinvc_t[:], dinvc_d[:])
            # node-block-major resident tables
            T1_s = tabp.tile([128, TS, F1], TAB_DT, tag="T1s")
            T2_s = tabp.tile([128, TS, F1], TAB_DT, tag="T2s")

            # ---- phase A: T1 = xT' @ W1 (sharded) ----
            t1_writes = []
            with tc.tile_pool(name="xwps", bufs=3, space="PSUM") as xpp:
                for (g, blocks, nrp) in groups:
                    r0 = g * 512
                    ncols = len(blocks) * 128
                    xt = xp.tile([128, KD, 512], TAB_DT, tag="xt")
                    nc.sync.dma_start(
                        xt[:, :, 0:ncols],
                        xT_d[:, :].rearrange("(k p) n -> p k n", p=128)
                        [:, :, r0:r0 + ncols])
                    ps = xpp.tile([128, 512], dt.float32, tag="xwps")
                    for si, b in enumerate(blocks):
                        for k in range(KD):
                            nc.tensor.matmul(
                                ps[:, si * 128:(si + 1) * 128],
                                xt[:, k, si * 128:(si + 1) * 128],
                                W1_t[:, k, :],
                                start=(k == 0), stop=(k == KD - 1))
                    nc.scalar.activation(
                        T1_s[:, blocks[0]:blocks[0] + len(blocks), :]
                        .rearrange("p s f -> p (s f)"),
                        ps[:, 0:ncols],
                        mybir.ActivationFunctionType.Copy)
                    w = nc.sync.dma_start(
                        T1_d[r0:r0 + len(blocks) * 128, :]
                        .rearrange("(p s) f -> p s f", s=nrp),
                        T1_s[:, blocks[0]:blocks[0] + len(blocks), :])
                    t1_writes.append(w)

            # metadata loads after the xw stream so T1 lands sooner
            idx_t = cp.tile([128, NCH * 8], dt.int16, tag="idx")
            nc.sync.dma_start(idx_t[:], idx_d[:])
            dstl_t = cp.tile([128, NCH], dt.float32, tag="dstl")
            nc.sync.dma_start(dstl_t[:], dstl_d[:])

            # ---- shared aggregation pass ----
            def agg_pass(tab_d, tab_writes, part, lyr, dve_after=None,
                         pe_after=None):
                rs_insts = [None, None]
                last_ops = {'pe': None, 'dve': None, 'act': None}
                block_done = {}
                p_writes = [[], []]
                # per-chunk tile slot bookkeeping
                win_of_chunk = {}
                for wi, (o, m) in enumerate(windows):
                    for k in range(o, o + m):
                        win_of_chunk[k] = (wi, k - o)
                with tc.tile_pool(name=f"aggps{lyr}", bufs=8, space="PSUM") as app:
                    gts = {}
                    accs = {}
                    evac_sel = 0
                    ch = 0
                    for wi, (o, m) in enumerate(windows):
                        gt = gp.tile([128, WINDOW, F1], TAB_DT, tag="gat")
                        gts[wi] = gt
                        gi = nc.gpsimd.dma_gather(
                            gt[:, 0:m, :], tab_d[:],
                            idx_t[:, o * 8:(o + m) * 8],
                            num_idxs=m * 128, num_idxs_reg=m * 128,
                            elem_size=F1)
                        for dep in tab_writes:
                            add_dep_helper(gi.ins, dep.ins, reason="table dep")
                        for k in range(o, o + m):
                            d, b = chunks[k]
                            grp = min(b // SGB, 12)
                            key = (d, grp)
                            if key not in accs:
                                accs[key] = app.tile([128, 512], dt.float32,
                                                     name=f"acc{lyr}_{d}_{grp}",
                                                     tag="acc")
                            acc = accs[key]
                            si = b - grp * SGB
                            oh = ohp.tile([128, 128], TAB_DT, tag="oh")
                            ohi = nc.vector.tensor_scalar(
                                oh[:], iota_t[:], dstl_t[:, k:k + 1], None,
                                mybir.AluOpType.is_equal)
                            if dve_after is not None:
                                add_dep_helper(ohi.ins, dve_after.ins,
                                               reason="order after prev phase")
                                dve_after = None
                            last_ops['dve'] = ohi
                            co, cm = cfg['chunk_of_block'][(d, b)]
                            if pe_after is not None:
                                pass
                            last_ops['pe'] = nc.tensor.matmul(
                                acc[:, si * 128:(si + 1) * 128],
                                oh[:], gt[:, k - o, :],
                                start=(k == co), stop=(k == co + cm - 1))
                            if k == co + cm - 1:
                                block_done.setdefault(key, []).append(b)
                                grp_blocks = groups[grp][1]
                                if len(block_done[key]) == len(grp_blocks):
                                    nrp = groups[grp][2]
                                    ncols = len(grp_blocks) * 128
                                    st = evp.tile([128, 512], PART_DT[lyr],
                                                  tag=f"pstage{lyr}")
                                    last_ops['act'] = nc.scalar.activation(
                                        st[:, 0:ncols], acc[:, 0:ncols],
                                        mybir.ActivationFunctionType.Copy)
                                    sl = 0 if grp < s0 else 1
                                    r0 = (grp * 512 if sl == 0
                                          else grp * 512 - srows[0])
                                    pw = nc.scalar.dma_start(
                                        part[sl][d * srows[sl] + r0:
                                                 d * srows[sl] + r0 + ncols, :]
                                        .rearrange("(p s) f -> p s f", s=nrp),
                                        st[:, 0:ncols]
                                        .rearrange("p (s f) -> p s f", s=nrp))
                                    p_writes[sl].append(pw)
                        # slice end -> fire RS
                        nxt = windows[wi + 1] if wi + 1 < len(windows) else None
                        cur_sl = cfg['slice_of_chunk'][o]
                        if nxt is None or cfg['slice_of_chunk'][nxt[0]] != cur_sl:
                            cc = nc.gpsimd.collective_compute(
                                "ReduceScatter", mybir.AluOpType.add,
                                replica_groups=[list(range(NCORES))],
                                ins=[part[cur_sl][:]],
                                outs=[agg_d[lyr][cur_sl][:]])
                            cc.ins.engine = mybir.EngineType.SP
                            for w in p_writes[cur_sl]:
                                add_dep_helper(cc.ins, w.ins,
                                               reason="rs reads partials")
                            rs_insts[cur_sl] = cc
                return rs_insts, last_ops

            rs1, l1_last = agg_pass(T1_d, t1_writes, p_d[0], 0)

            # ---- phase C: agg1 -> T2 ----
            # per-slice block-major loads of the RS output: full 512-row
            # groups land via (g p s) -> p (g s); the leftover 128-row group
            # is a separate 1-block dma.
            def load_agg_slices(agg, rs, tag):
                tiles = {}
                adt = agg[0].dtype if hasattr(agg[0], 'dtype') else TAB_DT
                for sl in range(2):
                    gfull = SLICE_GROUPS[sl] - (1 if sl == 1 else 0)
                    raw = tabp.tile([128, gfull, SGB, F1], adt,
                                    tag=f"{tag}r{sl}")
                    ld = nc.sync.dma_start(
                        raw[:],
                        agg[sl][0:gfull * 512, :]
                        .rearrange("(g p s) f -> p g s f", p=128, s=SGB))
                    add_dep_helper(ld.ins, rs[sl].ins, reason="load after rs")
                    if adt != TAB_DT:
                        t = tabp.tile([128, gfull, SGB, F1], TAB_DT,
                                      tag=f"{tag}{sl}")
                        nc.scalar.activation(
                            t[:].rearrange("p g s f -> p (g s f)"),
                            raw[:].rearrange("p g s f -> p (g s f)"),
                            mybir.ActivationFunctionType.Copy)
                    else:
                        t = raw
                    tl = None
                    if sl == 1:
                        rawl = tabp.tile([128, 1, F1], adt, tag=f"{tag}rL")
                        ld2 = nc.sync.dma_start(
                            rawl[:],
                            agg[sl][gfull * 512:gfull * 512 + 128, :]
                            .rearrange("(p s) f -> p s f", s=1))
                        add_dep_helper(ld2.ins, rs[sl].ins,
                                       reason="load after rs")
                        if adt != TAB_DT:
                            tl = tabp.tile([128, 1, F1], TAB_DT,
                                           tag=f"{tag}L")
                            nc.scalar.activation(
                                tl[:].rearrange("p s f -> p (s f)"),
                                rawl[:].rearrange("p s f -> p (s f)"),
                                mybir.ActivationFunctionType.Copy)
                        else:
                            tl = rawl
                    tiles[sl] = (t, tl)

                def blk_ap(b):
                    sl = 0 if b < s0 * SGB else 1
                    off = b - (0 if sl == 0 else s0 * SGB)
                    t, tl = tiles[sl]
                    if sl == 1 and off == (SLICE_GROUPS[1] - 1) * SGB:
                        return tl[:, 0, :]
                    return t[:, off // SGB, off % SGB, :]
                return blk_ap

            t2_writes = []
            with tc.tile_pool(name="cps", bufs=4, space="PSUM") as cpp:
                blk1 = load_agg_slices(agg_d[0], rs1, "agg1s")
                for (g, blocks, nrp) in groups:
                    ncols = len(blocks) * 128
                    for si, b in enumerate(blocks):
                        ps = cpp.tile([128, F1], dt.float32, tag="cps")
                        q = ps[:]
                        m1 = nc.tensor.matmul(q, ident_t[:], blk1(b),
                                              start=True, stop=False)
                        add_dep_helper(m1.ins, l1_last['pe'].ins,
                                       reason="order after L1 PE tail")
                        nc.tensor.matmul(q, ident_t[:], T1_s[:, b, :],
                                         start=False, stop=False)
                        nc.tensor.matmul(q,
                                         invd_t[:, b * 128:(b + 1) * 128],
                                         b1r_t[:], start=False, stop=True)
                        sc = nc.scalar.activation(
                            T2_s[:, b, :], q,
                            mybir.ActivationFunctionType.Relu,
                            scale=dinv2_t[:, b:b + 1])
                        add_dep_helper(sc.ins, l1_last['act'].ins,
                                       reason="order after L1 ACT tail")
                        c_last_dve = sc
                    w = nc.scalar.dma_start(
                        T2_d[g * 512:g * 512 + ncols, :]
                        .rearrange("(p s) f -> p s f", s=nrp),
                        T2_s[:, blocks[0]:blocks[0] + len(blocks), :])
                    t2_writes.append(w)

            rs2, l2_last = agg_pass(T2_d, t2_writes, p_d[1], 1,
                                    dve_after=c_last_dve)

            # ---- phase E: out = (dinv*(agg2 + T2self)) @ W2 + b2 ----
            with (
                tc.tile_pool(name="eps", bufs=2, space="PSUM") as epp,
                tc.tile_pool(name="ops", bufs=4, space="PSUM") as opp,
            ):
                blk2 = load_agg_slices(agg_d[1], rs2, "agg2s")
                for (g, blocks, nrp) in groups:
                    ncols = len(blocks) * 128
                    ost = evp.tile([F2, SGB, 128], dt.float32, tag="ostage")
                    for si, b in enumerate(blocks):
                        ps = epp.tile([128, F1], dt.float32, tag="eps")
                        q = ps[:]
                        m1 = nc.tensor.matmul(q, ident_t[:], blk2(b),
                                              start=True, stop=False)
                        add_dep_helper(m1.ins, l2_last['pe'].ins,
                                       reason="order after L2 PE tail")
                        nc.tensor.matmul(q, ident_t[:], T2_s[:, b, :],
                                         start=False, stop=True)
                        z = pp.tile([128, F1], TAB_DT, tag="z")
                        zo = nc.scalar.activation(
                            z[:], q, mybir.ActivationFunctionType.Identity,
                            scale=dinvc_t[:, b:b + 1])
                        add_dep_helper(zo.ins, l2_last['act'].ins,
                                       reason="order after L2 ACT tail")
                        pt = epp.tile([128, 128], TAB_DT, tag="psT")
                        nc.tensor.transpose(pt[:], z[:], ident_t[:])
                        zT = pp.tile([F1, 128], TAB_DT, tag="zT")
                        nc.vector.tensor_copy(zT[:], pt[:])
                        po = opp.tile([F2, 128], dt.float32, tag="ops")
                        nc.tensor.matmul(po[:], W2_t[:], zT[:],
                                         start=True, stop=True)
                        nc.scalar.activation(
                            ost[:, si, :], po[:],
                            mybir.ActivationFunctionType.Identity,
                            bias=b2_t[:, 0:1], scale=1.0)
                    nc.scalar.dma_start(
                        out_d[:, g * 512:g * 512 + ncols]
                        .rearrange("f (s n) -> f s n", s=nrp),
                        ost[:, 0:len(blocks), :])

    if not nc.is_finalized():
        nc.finalize()
    hoist_excess_waits(nc)
    return nc


# ---------------------------------------------------------------------------
cfg = None  # set by _kernel_impl for _build's closure use


def _kernel_impl(x, edge_index, W1, b1, W2, b2, ncores=NCORES):
    global cfg
    x = np.asarray(x, dtype=np.float32)
    edge_index = np.asarray(edge_index)
    W1 = np.asarray(W1, dtype=np.float32)
    b1 = np.asarray(b1, dtype=np.float32)
    W2 = np.asarray(W2, dtype=np.float32)
    b2 = np.asarray(b2, dtype=np.float32)
    N, D = x.shape
    F1 = W1.shape[1]
    F2 = W2.shape[1]

    cfg = _prepare(x, edge_index, ncores)
    nc = _build(cfg, F1, F2)

    tabnp = _np_dt(TAB_DT)
    in_maps = []
    for c in range(ncores):
        in_maps.append({
            "xT": cfg['xT_np'][c].astype(tabnp),
            "W1": W1.astype(tabnp),
            "b1r": b1.reshape(1, F1).astype(tabnp),
            "W2": W2.astype(tabnp),
            "b2": b2.reshape(F2, 1).astype(np.float32),
            "iota": cfg['iota'].astype(tabnp),
            "ident": cfg['ident'].astype(tabnp),
            "invd": cfg['invdr'][c].astype(tabnp),
            "dinv2c": cfg['dinv2c'][c],
            "dinvc": cfg['dinvc'][c],
            "idx": cfg['idx_np'][c],
            "dstl": cfg['dstl_np'][c],
        })
    res = run_bass_kernel_spmd(nc, in_maps, list(range(ncores)))

    out = np.empty((N, F2), np.float32)
    for c in range(ncores):
        oT = res.results[c]["outT"]          # [F2, NSHP], block-major cols
        n0, n1 = c * NSH, min(N, (c + 1) * NSH)
        cols = cfg['node_slot'][n0:n1]
        out[n0:n1] = oT[:, cols].T
    return out, res, nc, cfg


def kernel(x, edge_index, W1, b1, W2, b2):
    out, _, _, _ = _kernel_impl(x, edge_index, W1, b1, W2, b2)
    return out

